# revision 19
# baseline (speedup 1.0000x reference)
"""Trainium2 Bass kernel for nn_Encoder: 4-layer dual-stream transformer.

Strategy: data-parallel over batch (B=16 -> 2 items per core x 8 cores).
On-chip layout is feature-major ("X^T"): activations live as [128, 6*T] SBUF
tiles where column = d_tile*T + token. All matmuls run with bf16 operands and
fp32 PSUM accumulation; residual streams and LN statistics stay fp32
(LN stat matmuls use float32r). Attention uses an S^T = [keys, queries]
score layout so no transposes are ever needed: softmax denominators come from
ones-vector matmuls and the division is applied to the (tiny) O^T via
gpsimd partition-broadcast reciprocals. Heads are packed in pairs onto the
128-wide PE array with tile_position (rows for QK^T, columns for PV).
"""
import sys
import numpy as np

sys.path.insert(0, "/opt/trn_rl_repo")

D, H, DK, NLAYERS = 768, 12, 64, 4
B, SV, SW = 16, 512, 128
NCORES = 8
BPC = B // NCORES          # batch items per core = 2
TV = BPC * SV              # v-stream tokens per core = 1024
TW = BPC * SW              # w/s-stream tokens per core = 256
EPS = 1e-5
NDT = D // 128             # 6 d-tiles

# (path, din, dout, is_q) in wblob order
LINS = [
    ("attn1.q", D, D, True), ("attn1.k", D, D, False), ("attn1.v", D, D, False), ("attn1.o", D, D, False),
    ("attn2.q", D, D, True), ("attn2.k", D, D, False), ("attn2.v", D, D, False), ("attn2.o", D, D, False),
    ("dual.attn1.q", D, D, True), ("dual.attn1.k", D, D, False), ("dual.attn1.v", D, D, False), ("dual.attn1.o", D, D, False),
    ("dual.attn2.q", D, D, True), ("dual.attn2.k", D, D, False), ("dual.attn2.v", D, D, False), ("dual.attn2.o", D, D, False),
    ("dual.attn3.q", D, D, True), ("dual.attn3.k", D, D, False), ("dual.attn3.v", D, D, False), ("dual.attn3.o", D, D, False),
    ("ff.fc1", D, 4 * D, False), ("ff.fc2", 4 * D, D, False),
    ("dual.ff1.fc1", D, 4 * D, False), ("dual.ff1.fc2", 4 * D, D, False),
    ("dual.ff2.fc1", D, 4 * D, False), ("dual.ff2.fc2", 4 * D, D, False),
]
LNS = ["ln1", "ln2", "ln3", "dual.ln1", "dual.ln2", "dual.ln3", "dual.ln4"]

W_OFF = []
_off = 0
for _, din, dout, _q in LINS:
    W_OFF.append(_off)
    _off += din * dout
WBLOB_ELEMS = _off


def _pblob_layout():
    """Column map of the per-partition fp32 param blob [128, ncols]."""
    cols = {}
    c = 0
    for L in range(NLAYERS):
        for li, (_, din, dout, _q) in enumerate(LINS):
            cols[(L, "b", li)] = c
            c += dout // 128
        for j in range(len(LNS)):
            cols[(L, "g", j)] = c
            c += NDT
            cols[(L, "lb", j)] = c
            c += NDT
    return cols, c


PCOLS, NPCOL = _pblob_layout()

LN_HAS_B = False     # set true when any LN bias is nonzero (emits extra pass)
VB_NONZERO = False   # set true when any V-projection bias is nonzero


def _get(d, path):
    for k in path.split("."):
        d = d[k]
    return d


def _build_program():
    import concourse.bass as bass
    import concourse.bacc as bacc
    import concourse.mybir as mybir
    from concourse import tile
    from concourse import tile_utils
    tile_utils.max_sbuf_usage = 208 * 1024

    F32 = mybir.dt.float32
    F32R = mybir.dt.float32r
    BF16 = mybir.dt.bfloat16
    AF = mybir.ActivationFunctionType
    ALU = mybir.AluOpType

    nc = bacc.Bacc("TRN2", target_bir_lowering=False, debug=False,
                   num_devices=NCORES)

    vT_in = nc.dram_tensor("vT", [NDT, 128, TV], F32, kind="ExternalInput")
    wT_in = nc.dram_tensor("wT", [NDT, 128, TW], F32, kind="ExternalInput")
    sT_in = nc.dram_tensor("sT", [NDT, 128, TW], F32, kind="ExternalInput")
    wblobs = [nc.dram_tensor(f"wb{L}", [WBLOB_ELEMS], BF16, kind="ExternalInput")
              for L in range(NLAYERS)]
    pblob_in = nc.dram_tensor("pblob", [128, NPCOL], F32, kind="ExternalInput")
    vT_out = nc.dram_tensor("vTo", [NDT, 128, TV], F32, kind="ExternalOutput")
    wT_out = nc.dram_tensor("wTo", [NDT, 128, TW], F32, kind="ExternalOutput")
    sT_out = nc.dram_tensor("sTo", [NDT, 128, TW], F32, kind="ExternalOutput")

    wviews = []
    for L in range(NLAYERS):
        vs = []
        for li, (_, din, dout, _q) in enumerate(LINS):
            ap = wblobs[L].ap()[W_OFF[li]:W_OFF[li] + din * dout]
            vs.append(ap.rearrange("(a p m) -> a p m", p=128, m=dout))
        wviews.append(vs)

    with tile.TileContext(nc) as tc:
        import contextlib
        ctx = contextlib.ExitStack()
        with ctx:
            cpool = ctx.enter_context(tc.tile_pool(name="cpool", bufs=1))
            vpool = ctx.enter_context(tc.tile_pool(name="vpool", bufs=1))
            wspool = ctx.enter_context(tc.tile_pool(name="wspool", bufs=2))
            x2vp = ctx.enter_context(tc.tile_pool(name="x2vp", bufs=2))
            x2wp = ctx.enter_context(tc.tile_pool(name="x2wp", bufs=2))
            cstp = ctx.enter_context(tc.tile_pool(name="cstp", bufs=1))
            lnbp = ctx.enter_context(tc.tile_pool(name="lnbp", bufs=8))
            lnfp = ctx.enter_context(tc.tile_pool(name="lnfp", bufs=2))
            bcp = ctx.enter_context(tc.tile_pool(name="bcp", bufs=1))
            abp = ctx.enter_context(tc.tile_pool(name="abp", bufs=1))
            ssp = ctx.enter_context(tc.tile_pool(name="ssp", bufs=1))
            qtp = ctx.enter_context(tc.tile_pool(name="qtp", bufs=2))
            ktp = ctx.enter_context(tc.tile_pool(name="ktp", bufs=2))
            vtp = ctx.enter_context(tc.tile_pool(name="vtp", bufs=2))
            otp = ctx.enter_context(tc.tile_pool(name="otp", bufs=2))
            exp_ = ctx.enter_context(tc.tile_pool(name="exp", bufs=3))
            hp_ = ctx.enter_context(tc.tile_pool(name="hp", bufs=6))
            fap = ctx.enter_context(tc.tile_pool(name="fap", bufs=1))
            wpp = ctx.enter_context(tc.tile_pool(name="wpp", bufs=6))
            wf1p = ctx.enter_context(tc.tile_pool(name="wf1p", bufs=6))
            wf2p = ctx.enter_context(tc.tile_pool(name="wf2p", bufs=6))
            pa = ctx.enter_context(tc.tile_pool(name="pa", bufs=2, space="PSUM"))
            pb = ctx.enter_context(tc.tile_pool(name="pb", bufs=2, space="PSUM"))
            pc = ctx.enter_context(tc.tile_pool(name="pc", bufs=1, space="PSUM"))

            pbt = cpool.tile([128, NPCOL], F32, tag="pblob")
            nc.sync.dma_start(pbt[:], pblob_in.ap())
            inv_db = cpool.tile([128, 1], BF16, tag="invd")
            nc.gpsimd.memset(inv_db[:], 1.0 / D)
            ones_b = cpool.tile([128, 1], BF16, tag="onesb")
            nc.gpsimd.memset(ones_b[:], 1.0)
            eps_c = cpool.tile([128, 1], F32, tag="epsc")
            nc.gpsimd.memset(eps_c[:], EPS)

            def bcol(L, li):
                return PCOLS[(L, "b", li)]

            vres = vpool.tile([128, NDT * TV], F32, tag="vres")
            for di in range(NDT):
                nc.sync.dma_start(vres[:, di * TV:(di + 1) * TV], vT_in.ap()[di])
            wres = wspool.tile([128, NDT * TW], F32, tag="wres")
            sres = wspool.tile([128, NDT * TW], F32, tag="sres")
            for di in range(NDT):
                nc.sync.dma_start(wres[:, di * TW:(di + 1) * TW], wT_in.ap()[di])
                nc.sync.dma_start(sres[:, di * TW:(di + 1) * TW], sT_in.ap()[di])

            def load_w(L, li):
                _, din, dout, _q = LINS[li]
                assert din == D
                tiles = []
                for di in range(NDT):
                    t = wpp.tile([128, dout], BF16, tag="wp")
                    nc.sync.dma_start(t[:], wviews[L][li][di])
                    tiles.append(t)
                return tiles

            def emit_ln(L, lnj, src, T, W, x2_out, fp32_out=None):
                """Feature-major layernorm. src fp32 [128, 6T] -> x2_out bf16.
                Stats via bf16 ones-matmuls (cast + square first); normalize as
                x2 = (x*g)*rstd_b + (-mean*rstd)_b*g [+ b].
                If fp32_out is given (may be src itself), also write the
                normalized fp32 value via the precise 3-pass path."""
                gc = PCOLS[(L, "g", lnj)]
                bc = PCOLS[(L, "lb", lnj)]
                nchunks = T // W
                for c in range(nchunks):
                    sl = lambda di: src[:, di * T + c * W: di * T + c * W + W]
                    cbs = []
                    for di in range(NDT):
                        cb = lnbp.tile([128, W], BF16, tag="lnb")
                        nc.scalar.copy(cb[:], sl(di))
                        cbs.append(cb)
                    sqs = []
                    for di in range(NDT):
                        sq = lnbp.tile([128, W], BF16, tag="lnb")
                        nc.vector.tensor_tensor(sq[:], cbs[di][:], cbs[di][:], ALU.mult)
                        sqs.append(sq)
                    m_ps = pb.tile([1, W], F32, tag="sc0")
                    q_ps = pb.tile([1, W], F32, tag="sc1")
                    for di in range(NDT):
                        nc.tensor.matmul(m_ps[:], lhsT=inv_db[:], rhs=cbs[di][:],
                                         start=di == 0, stop=di == NDT - 1)
                    for di in range(NDT):
                        nc.tensor.matmul(q_ps[:], lhsT=inv_db[:], rhs=sqs[di][:],
                                         start=di == 0, stop=di == NDT - 1)
                    mean = ssp.tile([1, W], F32, tag="s0")
                    nc.vector.tensor_copy(mean[:], m_ps[:])
                    var = ssp.tile([1, W], F32, tag="s1")
                    nc.vector.tensor_tensor(var[:], m_ps[:], mean[:], ALU.mult)
                    nc.vector.tensor_tensor(var[:], q_ps[:], var[:], ALU.subtract)
                    nc.scalar.activation(var[:], var[:], AF.Sqrt, bias=eps_c[0:1, :])
                    rstd = ssp.tile([1, W], F32, tag="s2")
                    nc.vector.reciprocal(rstd[:], var[:])
                    r_b = bcp.tile([128, W], F32, tag="rb")
                    nc.gpsimd.partition_broadcast(r_b[:], rstd[:])
                    if fp32_out is None:
                        nmr = ssp.tile([1, W], F32, tag="s1")
                        nc.vector.tensor_tensor(nmr[:], mean[:], rstd[:], ALU.mult)
                        nc.vector.tensor_scalar_mul(nmr[:], nmr[:], -1.0)
                        n_b = bcp.tile([128, W], F32, tag="nb")
                        nc.gpsimd.partition_broadcast(n_b[:], nmr[:])
                        for di in range(NDT):
                            g_ap = pbt[:, gc + di:gc + di + 1]
                            xsl = x2_out[:, di * T + c * W: di * T + c * W + W]
                            u = lnbp.tile([128, W], BF16, tag="lnb")
                            nc.vector.scalar_tensor_tensor(
                                u[:], sl(di), g_ap, r_b[:], ALU.mult, ALU.mult)
                            nc.vector.scalar_tensor_tensor(
                                xsl, n_b[:], g_ap, u[:], ALU.mult, ALU.add)
                            if LN_HAS_B:
                                nc.vector.tensor_scalar_add(
                                    xsl, xsl, pbt[:, bc + di:bc + di + 1])
                    else:
                        m_b = bcp.tile([128, W], F32, tag="mb")
                        nc.gpsimd.partition_broadcast(m_b[:], mean[:])
                        for di in range(NDT):
                            g_ap = pbt[:, gc + di:gc + di + 1]
                            t = lnfp.tile([128, W], F32, tag="lnf")
                            nc.vector.tensor_tensor(t[:], sl(di), m_b[:], ALU.subtract)
                            nc.vector.scalar_tensor_tensor(
                                t[:], t[:], g_ap, r_b[:], ALU.mult, ALU.mult)
                            nc.scalar.activation(
                                x2_out[:, di * T + c * W: di * T + c * W + W], t[:],
                                AF.Identity, bias=pbt[:, bc + di:bc + di + 1])
                            if LN_HAS_B:
                                nc.vector.tensor_scalar_add(
                                    fp32_out[:, di * T + c * W: di * T + c * W + W],
                                    t[:], pbt[:, bc + di:bc + di + 1])
                            else:
                                nc.vector.tensor_copy(
                                    fp32_out[:, di * T + c * W: di * T + c * W + W],
                                    t[:])

            def emit_projF(src, T, c, W, wt, bc_, out, ocol):
                """Feature-major projection: out[:, ocol(do):+W] (bf16)."""
                for do in range(NDT):
                    ps = pa.tile([128, W], F32, tag="ps")
                    for di in range(NDT):
                        nc.tensor.matmul(
                            ps[:], lhsT=wt[di][:, do * 128:(do + 1) * 128],
                            rhs=src[:, di * T + c * W: di * T + c * W + W],
                            start=di == 0, stop=di == NDT - 1)
                    nc.scalar.activation(out[:, ocol(do): ocol(do) + W], ps[:],
                                         AF.Identity,
                                         bias=pbt[:, bc_ + do:bc_ + do + 1])

            def emit_projT(src, T, it, kvlen, wt, vt):
                """Token-major V projection for one item: vt [128, ntt*768]."""
                for tt in range(kvlen // 128):
                    tok0 = it * kvlen + tt * 128
                    ps1 = pa.tile([128, 512], F32, tag="ps")
                    ps2 = pa.tile([128, 256], F32, tag="ps")
                    for di in range(NDT):
                        lhs = src[:, di * T + tok0: di * T + tok0 + 128]
                        nc.tensor.matmul(ps1[:], lhsT=lhs, rhs=wt[di][:, 0:512],
                                         start=di == 0, stop=di == NDT - 1)
                    for di in range(NDT):
                        lhs = src[:, di * T + tok0: di * T + tok0 + 128]
                        nc.tensor.matmul(ps2[:], lhsT=lhs, rhs=wt[di][:, 512:768],
                                         start=di == 0, stop=di == NDT - 1)
                    nc.scalar.activation(vt[:, tt * 768: tt * 768 + 512], ps1[:], AF.Identity)
                    nc.scalar.activation(vt[:, tt * 768 + 512: tt * 768 + 768], ps2[:], AF.Identity)

            def emit_projR(ot, OW, c, wt, bc_, resid, T):
                """O-projection + residual add into resid chunk c (in place or
                into a different destination AP of same layout)."""
                for do in range(NDT):
                    ps = pa.tile([128, OW], F32, tag="ps")
                    for di in range(NDT):
                        nc.tensor.matmul(
                            ps[:], lhsT=wt[di][:, do * 128:(do + 1) * 128],
                            rhs=ot[:, di * OW:(di + 1) * OW],
                            start=di == 0, stop=di == NDT - 1)
                    dst = resid[:, do * T + c * OW: do * T + c * OW + OW]
                    nc.vector.scalar_tensor_tensor(
                        dst, ps[:], pbt[:, bc_ + do:bc_ + do + 1], dst,
                        ALU.add, ALU.add)

            def emit_attn(qt, kt, vt, ot, q_len, kv_len, qoff, koff, ooff, vbc=None):
                kt_n = kv_len // 128
                for hpi in range(NDT):
                    den = pc.tile([64, q_len], F32, tag="d")
                    ops = pc.tile([128, q_len], F32, tag="o")
                    for k_ in range(kt_n):
                        s0 = pb.tile([128, q_len], F32, tag="sc0")
                        s1 = pb.tile([128, q_len], F32, tag="sc1")
                        nc.tensor.matmul(
                            s0[:], lhsT=kt[0:64, koff(hpi) + k_ * 128: koff(hpi) + k_ * 128 + 128],
                            rhs=qt[0:64, qoff(hpi): qoff(hpi) + q_len],
                            start=True, stop=True, tile_position=(0, 0))
                        nc.tensor.matmul(
                            s1[:], lhsT=kt[64:128, koff(hpi) + k_ * 128: koff(hpi) + k_ * 128 + 128],
                            rhs=qt[64:128, qoff(hpi): qoff(hpi) + q_len],
                            start=True, stop=True, tile_position=(64, 0))
                        e0 = exp_.tile([128, q_len], BF16, tag="e")
                        e1 = exp_.tile([128, q_len], BF16, tag="e")
                        nc.scalar.activation(e0[:], s0[:], AF.Exp)
                        nc.scalar.activation(e1[:], s1[:], AF.Exp)
                        first, last = k_ == 0, k_ == kt_n - 1
                        nc.tensor.matmul(den[0:1, :], lhsT=ones_b[:], rhs=e0[:],
                                         start=first, stop=last, tile_position=(0, 0))
                        nc.tensor.matmul(den[32:33, :], lhsT=ones_b[:], rhs=e1[:],
                                         start=first, stop=last, tile_position=(0, 32))
                        nc.tensor.matmul(
                            ops[0:64, :], lhsT=vt[:, k_ * 768 + 2 * hpi * 64: k_ * 768 + 2 * hpi * 64 + 64],
                            rhs=e0[:], start=first, stop=last, tile_position=(0, 0))
                        nc.tensor.matmul(
                            ops[64:128, :], lhsT=vt[:, k_ * 768 + (2 * hpi + 1) * 64: k_ * 768 + (2 * hpi + 1) * 64 + 64],
                            rhs=e1[:], start=first, stop=last, tile_position=(0, 64))
                    r0 = ssp.tile([1, q_len], F32, tag="s0")
                    r1 = ssp.tile([1, q_len], F32, tag="s1")
                    nc.vector.reciprocal(r0[:], den[0:1, :])
                    nc.vector.reciprocal(r1[:], den[32:33, :])
                    rb0 = abp.tile([64, q_len], F32, tag="rb0")
                    rb1 = abp.tile([64, q_len], F32, tag="rb1")
                    nc.gpsimd.partition_broadcast(rb0[:], r0[:])
                    nc.gpsimd.partition_broadcast(rb1[:], r1[:])
                    o_sl = ot[:, ooff(hpi): ooff(hpi) + q_len]
                    nc.vector.tensor_tensor(ot[0:64, ooff(hpi): ooff(hpi) + q_len],
                                            ops[0:64, :], rb0[:], ALU.mult)
                    nc.vector.tensor_tensor(ot[64:128, ooff(hpi): ooff(hpi) + q_len],
                                            ops[64:128, :], rb1[:], ALU.mult)
                    if VB_NONZERO and vbc is not None:
                        nc.vector.tensor_scalar_add(
                            o_sl, o_sl, pbt[:, vbc + hpi:vbc + hpi + 1])

            def emit_ffn(L, li1, li2, x2, T, W, src_resid, dst_resid):
                """x2 bf16 [128,6T] -> hidden(gelu) -> fc2 -> dst = src + out."""
                b1c = bcol(L, li1)
                b2c = bcol(L, li2)
                v1 = wviews[L][li1]   # [6, 128, 3072]
                v2 = wviews[L][li2]   # [24, 128, 768]
                for c in range(T // W):
                    facc = fap.tile([128, NDT * W], F32, tag="facc")
                    for hb in range(4):
                        f1t = []
                        for di in range(NDT):
                            t = wf1p.tile([128, 768], BF16, tag="wf1")
                            nc.sync.dma_start(t[:], v1[di][:, hb * 768:(hb + 1) * 768])
                            f1t.append(t)
                        hts = []
                        for j in range(6):
                            ho = hb * 6 + j
                            ps = pa.tile([128, W], F32, tag="ps")
                            for di in range(NDT):
                                nc.tensor.matmul(
                                    ps[:], lhsT=f1t[di][:, j * 128:(j + 1) * 128],
                                    rhs=x2[:, di * T + c * W: di * T + c * W + W],
                                    start=di == 0, stop=di == NDT - 1)
                            ht = hp_.tile([128, W], BF16, tag="h")
                            nc.scalar.activation(ht[:], ps[:], AF.Gelu,
                                                 bias=pbt[:, b1c + ho:b1c + ho + 1])
                            hts.append(ht)
                        f2t = []
                        for j in range(6):
                            t = wf2p.tile([128, 768], BF16, tag="wf2")
                            nc.sync.dma_start(t[:], v2[hb * 6 + j])
                            f2t.append(t)
                        for do in range(NDT):
                            ps = pa.tile([128, W], F32, tag="ps")
                            for j in range(6):
                                nc.tensor.matmul(
                                    ps[:], lhsT=f2t[j][:, do * 128:(do + 1) * 128],
                                    rhs=hts[j][:],
                                    start=j == 0, stop=j == 5)
                            fa = facc[:, do * W:(do + 1) * W]
                            if hb == 0:
                                nc.vector.tensor_copy(fa, ps[:])
                            else:
                                nc.vector.tensor_tensor(fa, ps[:], fa, ALU.add)
                    for do in range(NDT):
                        dst = dst_resid[:, do * T + c * W: do * T + c * W + W]
                        src = src_resid[:, do * T + c * W: do * T + c * W + W]
                        nc.vector.scalar_tensor_tensor(
                            dst, facc[:, do * W:(do + 1) * W],
                            pbt[:, b2c + do:b2c + do + 1], src, ALU.add, ALU.add)

            def cast_bf16(src, T):
                t = cstp.tile([128, NDT * T], BF16, tag="cst")
                for di in range(NDT):
                    nc.scalar.copy(t[:, di * T:(di + 1) * T], src[:, di * T:(di + 1) * T])
                return t

            # ================= layers =================
            for L in range(NLAYERS):
                # --- v: ln1 + self attention ---
                x2v1 = x2vp.tile([128, NDT * TV], BF16, tag="x2v")
                emit_ln(L, 0, vres, TV, 512, x2v1)
                qts, kts, vts, ots = [], [], [], []
                wq = load_w(L, 0)
                for it in range(BPC):
                    qt = qtp.tile([128, NDT * 512], BF16, tag="qt")
                    emit_projF(x2v1, TV, it, 512, wq, bcol(L, 0), qt, lambda do: do * 512)
                    qts.append(qt)
                wk = load_w(L, 1)
                for it in range(BPC):
                    kt = ktp.tile([128, NDT * 512], BF16, tag="kt")
                    emit_projF(x2v1, TV, it, 512, wk, bcol(L, 1), kt, lambda do: do * 512)
                    kts.append(kt)
                wv_ = load_w(L, 2)
                for it in range(BPC):
                    vt = vtp.tile([128, 4 * 768], BF16, tag="vt")
                    emit_projT(x2v1, TV, it, 512, wv_, vt)
                    vts.append(vt)
                for it in range(BPC):
                    ot = otp.tile([128, NDT * 512], BF16, tag="ot")
                    emit_attn(qts[it], kts[it], vts[it], ot, 512, 512,
                              qoff=lambda hpi: hpi * 512,
                              koff=lambda hpi: hpi * 512,
                              ooff=lambda hpi: hpi * 512)
                    ots.append(ot)
                wo = load_w(L, 3)
                for it in range(BPC):
                    emit_projR(ots[it], 512, it, wo, bcol(L, 3), vres, TV)
                # --- ln2 -> e_in ---
                ein = x2vp.tile([128, NDT * TV], BF16, tag="x2v")
                emit_ln(L, 1, vres, TV, 512, ein)
                # ================= dual =================
                # d.ln1: w = ln(w_feat) (fp32, in place) + bf16
                x2w1 = x2wp.tile([128, NDT * TW], BF16, tag="x2w")
                emit_ln(L, 3, wres, TW, TW, x2w1, fp32_out=wres)
                # d.attn1 self on w
                wq1 = load_w(L, 8)
                qtw = qtp.tile([128, NDT * TW], BF16, tag="qt")
                emit_projF(x2w1, TW, 0, TW, wq1, bcol(L, 8), qtw, lambda do: do * TW)
                wk1 = load_w(L, 9)
                ktw = ktp.tile([128, NDT * TW], BF16, tag="kt")
                emit_projF(x2w1, TW, 0, TW, wk1, bcol(L, 9), ktw, lambda do: do * TW)
                wv1 = load_w(L, 10)
                vtws = []
                for it in range(BPC):
                    vtw = vtp.tile([128, 768], BF16, tag="vt")
                    emit_projT(x2w1, TW, it, 128, wv1, vtw)
                    vtws.append(vtw)
                otw = otp.tile([128, NDT * TW], BF16, tag="ot")
                for it in range(BPC):
                    vtw = vtws[it]
                    emit_attn(qtw, ktw, vtw, otw, 128, 128,
                              qoff=lambda hpi: hpi * TW + it * 128,
                              koff=lambda hpi: hpi * TW + it * 128,
                              ooff=lambda hpi: hpi * TW + it * 128)
                wo1 = load_w(L, 11)
                emit_projR(otw, TW, 0, wo1, bcol(L, 11), wres, TW)   # x_a
                # d.ln2
                x2w2 = x2wp.tile([128, NDT * TW], BF16, tag="x2w")
                emit_ln(L, 4, wres, TW, TW, x2w2)
                # d.attn2: q from w (128/it), kv from e_in (512/it)
                wq2 = load_w(L, 12)
                qtw2 = qtp.tile([128, NDT * TW], BF16, tag="qt")
                emit_projF(x2w2, TW, 0, TW, wq2, bcol(L, 12), qtw2, lambda do: do * TW)
                wk2 = load_w(L, 13)
                kt2s = []
                for it in range(BPC):
                    kt2 = ktp.tile([128, NDT * 512], BF16, tag="kt")
                    emit_projF(ein, TV, it, 512, wk2, bcol(L, 13), kt2, lambda do: do * 512)
                    kt2s.append(kt2)
                wv2 = load_w(L, 14)
                vt2s = []
                for it in range(BPC):
                    vt2 = vtp.tile([128, 4 * 768], BF16, tag="vt")
                    emit_projT(ein, TV, it, 512, wv2, vt2)
                    vt2s.append(vt2)
                otw2 = otp.tile([128, NDT * TW], BF16, tag="ot")
                for it in range(BPC):
                    kt2, vt2 = kt2s[it], vt2s[it]
                    emit_attn(qtw2, kt2, vt2, otw2, 128, 512,
                              qoff=lambda hpi: hpi * TW + it * 128,
                              koff=lambda hpi: hpi * 512,
                              ooff=lambda hpi: hpi * TW + it * 128)
                wo2 = load_w(L, 15)
                emit_projR(otw2, TW, 0, wo2, bcol(L, 15), wres, TW)  # x_b
                # d.ln3
                x2w3 = x2wp.tile([128, NDT * TW], BF16, tag="x2w")
                emit_ln(L, 5, wres, TW, TW, x2w3)
                # d.ff1 -> wout (new w stream)
                wout = wspool.tile([128, NDT * TW], F32, tag="wres")
                emit_ffn(L, 22, 23, x2w3, TW, TW, wres, wout)
                # d.attn3: q from x2w3, kv = s
                sbf = cast_bf16(sres, TW)
                wq3 = load_w(L, 16)
                qtw3 = qtp.tile([128, NDT * TW], BF16, tag="qt")
                emit_projF(x2w3, TW, 0, TW, wq3, bcol(L, 16), qtw3, lambda do: do * TW)
                wk3 = load_w(L, 17)
                ktw3 = ktp.tile([128, NDT * TW], BF16, tag="kt")
                emit_projF(sbf, TW, 0, TW, wk3, bcol(L, 17), ktw3, lambda do: do * TW)
                wv3 = load_w(L, 18)
                vtw3s = []
                for it in range(BPC):
                    vtw3 = vtp.tile([128, 768], BF16, tag="vt")
                    emit_projT(sbf, TW, it, 128, wv3, vtw3)
                    vtw3s.append(vtw3)
                otw3 = otp.tile([128, NDT * TW], BF16, tag="ot")
                for it in range(BPC):
                    vtw3 = vtw3s[it]
                    emit_attn(qtw3, ktw3, vtw3, otw3, 128, 128,
                              qoff=lambda hpi: hpi * TW + it * 128,
                              koff=lambda hpi: hpi * TW + it * 128,
                              ooff=lambda hpi: hpi * TW + it * 128)
                wo3 = load_w(L, 19)
                emit_projR(otw3, TW, 0, wo3, bcol(L, 19), wres, TW)  # x_c = e_out
                # d.ln4 + ff2 -> sout
                x2w4 = x2wp.tile([128, NDT * TW], BF16, tag="x2w")
                emit_ln(L, 6, wres, TW, TW, x2w4)
                sout = wspool.tile([128, NDT * TW], F32, tag="sres")
                emit_ffn(L, 24, 25, x2w4, TW, TW, wres, sout)
                # ================= back to v =================
                ebf = cast_bf16(wres, TW)   # e_out bf16
                wka = load_w(L, 5)
                kta = ktp.tile([128, NDT * TW], BF16, tag="kt")
                emit_projF(ebf, TW, 0, TW, wka, bcol(L, 5), kta, lambda do: do * TW)
                wva = load_w(L, 6)
                vtas = []
                for it in range(BPC):
                    vta = vtp.tile([128, 768], BF16, tag="vt")
                    emit_projT(ebf, TW, it, 128, wva, vta)
                    vtas.append(vta)
                wqa = load_w(L, 4)
                qtas = []
                for it in range(BPC):
                    qta = qtp.tile([128, NDT * 512], BF16, tag="qt")
                    emit_projF(ein, TV, it, 512, wqa, bcol(L, 4), qta, lambda do: do * 512)
                    qtas.append(qta)
                otsa = []
                for it in range(BPC):
                    vta, qta = vtas[it], qtas[it]
                    ota = otp.tile([128, NDT * 512], BF16, tag="ot")
                    emit_attn(qta, kta, vta, ota, 512, 128,
                              qoff=lambda hpi: hpi * 512,
                              koff=lambda hpi: hpi * TW + it * 128,
                              ooff=lambda hpi: hpi * 512)
                    otsa.append(ota)
                woa = load_w(L, 7)
                for it in range(BPC):
                    emit_projR(otsa[it], 512, it, woa, bcol(L, 7), vres, TV)
                # ln3 + ff
                x2v3 = x2vp.tile([128, NDT * TV], BF16, tag="x2v")
                emit_ln(L, 2, vres, TV, 512, x2v3)
                emit_ffn(L, 20, 21, x2v3, TV, 512, vres, vres)
                wres = wout
                sres = sout

            for di in range(NDT):
                nc.sync.dma_start(vT_out.ap()[di], vres[:, di * TV:(di + 1) * TV])
                nc.sync.dma_start(wT_out.ap()[di], wres[:, di * TW:(di + 1) * TW])
                nc.sync.dma_start(sT_out.ap()[di], sres[:, di * TW:(di + 1) * TW])

    nc.compile()
    return nc


_PROGRAM = None


def _get_program():
    global _PROGRAM
    if _PROGRAM is None:
        _PROGRAM = _build_program()
    return _PROGRAM


def _pack_params(params):
    import ml_dtypes
    wblobs = []
    pblob = np.zeros((128, NPCOL), np.float32)
    for L in range(NLAYERS):
        p = params[L]
        parts = []
        for li, (path, din, dout, is_q) in enumerate(LINS):
            lin = _get(p, path)
            W = np.asarray(lin["W"], np.float32)
            b = np.asarray(lin["b"], np.float32)
            if is_q:
                W = W * 0.125
                b = b * 0.125
            parts.append(np.asarray(W, dtype=ml_dtypes.bfloat16).reshape(-1))
            c = PCOLS[(L, "b", li)]
            pblob[:, c:c + dout // 128] = b.reshape(dout // 128, 128).T
        for j, lname in enumerate(LNS):
            ln = _get(p, lname)
            g = np.asarray(ln["g"], np.float32)
            lb = np.asarray(ln["b"], np.float32)
            c = PCOLS[(L, "g", j)]
            pblob[:, c:c + NDT] = g.reshape(NDT, 128).T
            c = PCOLS[(L, "lb", j)]
            pblob[:, c:c + NDT] = lb.reshape(NDT, 128).T
        wblobs.append(np.concatenate(parts))
    return wblobs, pblob


def _set_flags(params):
    global LN_HAS_B, VB_NONZERO
    for L in range(NLAYERS):
        for lname in LNS:
            if np.any(np.asarray(_get(params[L], lname)["b"]) != 0):
                LN_HAS_B = True
        for path, _din, _dout, _q in LINS:
            if path.endswith(".v") and np.any(np.asarray(_get(params[L], path)["b"]) != 0):
                VB_NONZERO = True


def kernel(v_feat, w_feat, s_feat, params):
    from concourse.bass_utils import run_bass_kernel_spmd
    _set_flags(params)
    nc = _get_program()
    v_feat = np.asarray(v_feat, np.float32)
    w_feat = np.asarray(w_feat, np.float32)
    s_feat = np.asarray(s_feat, np.float32)
    wblobs, pblob = _pack_params(params)

    def featmaj(x, T):
        # [BPC, S, D] -> [NDT, 128, BPC*S]
        n = x.shape[1]
        t = x.transpose(2, 0, 1).reshape(D, BPC * n)
        return np.ascontiguousarray(t.reshape(NDT, 128, BPC * n))

    in_maps = []
    for i in range(NCORES):
        m = {
            "vT": featmaj(v_feat[BPC * i:BPC * (i + 1)], TV),
            "wT": featmaj(w_feat[BPC * i:BPC * (i + 1)], TW),
            "sT": featmaj(s_feat[BPC * i:BPC * (i + 1)], TW),
            "pblob": pblob,
        }
        for L in range(NLAYERS):
            m[f"wb{L}"] = wblobs[L]
        in_maps.append(m)

    res = run_bass_kernel_spmd(nc, in_maps, list(range(NCORES)))

    def unfeat(a, n):
        # [NDT, 128, BPC*n] -> [BPC, n, D]
        t = a.reshape(D, BPC, n)
        return t.transpose(1, 2, 0)

    v_out = np.concatenate([unfeat(res.results[i]["vTo"], SV) for i in range(NCORES)])
    w_out = np.concatenate([unfeat(res.results[i]["wTo"], SW) for i in range(NCORES)])
    s_out = np.concatenate([unfeat(res.results[i]["sTo"], SW) for i in range(NCORES)])
    return (np.ascontiguousarray(v_out), np.ascontiguousarray(w_out),
            np.ascontiguousarray(s_out))
